# revision 12
# baseline (speedup 1.0000x reference)
"""Trainium2 Bass kernel for nn_CellSmooth.

Computes: out = softmax(-cdist(enc, enc) + quality^T, axis=-1) @ expression
for B=1, N=8192, G=2048, D=64, sharded row-wise across 8 NeuronCores.

Design (per core, owning a 1024-row block of queries i):
  * d2[j, i] = |e_j|^2 + |e_i|^2 - 2 e_j.e_i is produced TRANSPOSED ([j, i]
    tiles, j on partitions) by a single K=66 augmented float32r matmul:
      U[:, j] = [enc_j (64), |e_j|^2, 1],  V[:, i] = [-2 enc_i (64), 1, |e_i|^2]
    U/V are built on the host (tiny). float32r runs the PE at full (bf16)
    rate for moving dims >= 256; measured accuracy ~7e-5 relative.
  * The d2_ii ~ 0 diagonal cannot survive float32r cancellation, so the host
    j-ROTATES the j-indexed inputs per core (roll by -1024*c): every core's
    diagonal then sits at compile-time-known j-tiles/positions (softmax's
    sum over j is permutation invariant, so the output is unchanged). Those
    positions are repaired after the exp with affine_select + a masked add
    of host-computed exp(quality) values.
  * P^T[j, i] = exp(quality_j - sqrt(d2)) is stored FP16 in 64 persistent
    [128, 1024] tiles covering the core's full 1024 i-rows (128 KB per
    partition): DVE relu-drains the d2 PSUM bank to the fp16 tile, ACT runs
    sqrt and exp in place (quality folds into the ACT exp bias). sqrt and
    exp live in different ACT table sets, so tiles go in groups of
    [sqrt x G, exp x G] to amortize the 1.3us table swaps. Measured fp16
    accuracy of the whole pipeline: 6e-4 relative.
  * The [j, i] P^T layout is exactly the stationary-operand layout the
    output matmul needs - the NxN matrix is never transposed. expression is
    converted to fp16 on the host: fp16 P x fp16 E matmuls run at the same
    1 cycle/row as f32r but halve SBUF and HBM traffic.
  * i is processed in two 512-column halves for phase 1 + g-block 0, each
    software-pipelined (d2/sqrt/exp groups chased two groups behind by the
    g-block-0 matmuls) so PE/ACT/DVE all stay busy; PSUM = 4 g0
    accumulators + 3-4 rotating d2 banks. denominator_i = sum_j P^T[j, i]
    accumulates on the otherwise-idle GPSIMD (acc += pt tile per j, after
    the diagonal fixup), then one f32 ones-stationary matmul reduces acc
    across partitions -> [1,512], redistributed to [128,4] through a DRAM
    bounce, then reciprocal. This keeps the PE free of 128 column-sum
    matmuls (~27us): on TRN2 every matmul pays a ~60ns stationary-load/
    drain overhead on top of its 512 moving cycles (measured 276ns vs the
    213ns compute for fp16, 253ns for f32r), which is why fewer, full-width
    matmuls win and why fp16's DMA savings roughly cancel f32r's cheaper
    self-loading stationaries elsewhere.
  * g-blocks 1..3 then run ONCE over the full 1024-wide P^T with all 8 PSUM
    banks as accumulators - expression streams from HBM once per g-block
    (fp16, 1KB/partition contiguous DMAs), 40 MB/core total vs 128 MB for
    the two-pass f32 version; out[i, g] = PSUM / den via DVE
    tensor_scalar multiply by 1/den, DMA out f32.
"""

import numpy as np

import concourse.bass as bass  # noqa: F401
import concourse.mybir as mybir
import concourse.tile as tile
from concourse import bacc

F32 = mybir.dt.float32
F32R = mybir.dt.float32r
F16 = mybir.dt.float16
AF = mybir.ActivationFunctionType
ALU = mybir.AluOpType

P = 128
N_CORES = 8


def build_nc(n=8192, d=64, rows=1024, g=2048, half=512, repeat=1, hw_loop=0,
             bounds_spec=(0, 4, 8, 16, 32, 48, 56, 64), out16=False, lag=2,
             direct_sqrt=False):
    """Build the per-core Bass program (SPMD: all per-core variation is in
    the input data, never in the instruction stream).

    repeat>1 re-runs the body unrolled; hw_loop>0 wraps the body in a
    hardware For_i loop (constant instruction count). Both are for measuring
    steady-state HW exec time by wall-clock differencing in test.py.
    """
    jt_n = n // P          # j tiles (contraction for the output matmul)
    n_half = rows // half  # i column passes for phase 1 + g-block 0
    it_n = half // P       # i tiles per half
    it_full = rows // P    # i tiles full-width
    gb_n = g // 512        # g blocks
    k = d + 2              # augmented contraction for the d2 matmul

    nc = bacc.Bacc(None, target_bir_lowering=False)
    u_d = nc.dram_tensor("u", [k, n], F32, kind="ExternalInput")
    v_d = nc.dram_tensor("v", [k, rows], F32, kind="ExternalInput")
    qt_d = nc.dram_tensor("qt", [P, jt_n], F32, kind="ExternalInput")
    eq_d = nc.dram_tensor("eq", [P, jt_n], F32, kind="ExternalInput")
    e_d = nc.dram_tensor("expr", [n, g], F16, kind="ExternalInput")
    o_d = nc.dram_tensor("out", [rows, g], F16 if out16 else F32,
                         kind="ExternalOutput")

    with tile.TileContext(nc) as tc:
        with (
            tc.tile_pool(name="const", bufs=1) as constp,
            tc.tile_pool(name="vpool", bufs=2) as vpool,
            tc.tile_pool(name="ptpool", bufs=1) as ptpool,
            tc.tile_pool(name="estream", bufs=6) as epool,
            tc.tile_pool(name="ostage", bufs=4) as opool,
            tc.tile_pool(name="small", bufs=2) as smallp,
            tc.tile_pool(name="dtmp", bufs=1) as dtmpp,
            tc.tile_pool(name="denacc", bufs=2) as daccp,
            tc.tile_pool(name="mmpsum", bufs=8, space="PSUM") as mmpsum,
            tc.tile_pool(name="scratch", bufs=2, space="DRAM") as dramp,
        ):
            u_sb = constp.tile([k, n], F32R, name="u_sb")
            nc.sync.dma_start(out=u_sb, in_=u_d[:, :].bitcast(F32R))
            qt_sb = constp.tile([P, jt_n], F32, name="qt_sb")
            nc.sync.dma_start(out=qt_sb, in_=qt_d[:, :])
            eq_sb = constp.tile([P, jt_n], F32, name="eq_sb")
            nc.sync.dma_start(out=eq_sb, in_=eq_d[:, :])
            ones_f32 = constp.tile([P, 1], F32, name="ones_f32")
            nc.vector.memset(ones_f32, 1.0)
            ones_sb = constp.tile([P, 1], F16, name="ones_sb")
            nc.vector.tensor_copy(out=ones_sb[:, :], in_=ones_f32[:, :])
            # Diagonal masks: m1[it][p, c] = 1.0 iff c == it*128 + p.
            dmask1 = []
            for it in range(it_n):
                m1 = constp.tile([P, half], F16, name=f"dm1_{it}")
                nc.gpsimd.memset(m1, 0.0)
                nc.gpsimd.affine_select(
                    out=m1, in_=m1, compare_op=ALU.not_equal, fill=1.0,
                    base=it * P, pattern=[[-1, half]], channel_multiplier=1,
                )
                dmask1.append(m1)

            def body():
                for _ in range(repeat):
                    # Persistent fp16 P^T tiles spanning the full 1024 i.
                    pt_t = [
                        ptpool.tile([P, rows], F16, name=f"pt{j}",
                                    tag=f"pt{j}")
                        for j in range(jt_n)
                    ]
                    recip_all = smallp.tile([P, it_full], F32,
                                            name="recip_all", tag="recip")

                    for h in range(n_half):
                        c0 = h * half
                        v_sb = vpool.tile([k, half], F32R, name="v_sb")
                        nc.sync.dma_start(
                            out=v_sb,
                            in_=v_d[:, c0:c0 + half].bitcast(F32R))

                        # ---- phase 1 software-pipelined with g-block 0 +
                        # den. The PE issues in program order, so a plain
                        # phase1-then-phase2 order leaves the PE slot-blocked
                        # behind the ACT sqrt/exp pipeline for most of
                        # phase 1. Instead, emit phase-1 groups interleaved
                        # with the g-block-0 and denominator matmuls of the
                        # previous group.
                        diag_lo, diag_hi = h * it_n, (h + 1) * it_n
                        # Small leading groups fill the ACT->PE pipeline
                        # sooner; larger tail groups amortize ACT table swaps.
                        bounds = list(bounds_spec) if jt_n == 64 \
                            else list(range(0, jt_n + 1, 8))
                        ps0_list = [
                            mmpsum.tile([P, 512], F32, name=f"ps0_{it}",
                                        tag="mm")
                            for it in range(it_n)
                        ]
                        # denominator accumulates on the idle GPSIMD
                        # (acc += pt tile per j), freeing the PE of 64
                        # column-sum matmuls; one f32 ones-matmul then
                        # reduces acc across partitions.
                        dacc = daccp.tile([P, half], F32, name="dacc")
                        nc.gpsimd.memset(dacc, 0.0)

                        def phase1_group(lo, hi):
                            for j in range(lo, hi):
                                d2 = mmpsum.tile([P, half], F32, name="d2",
                                                 tag="mm")
                                nc.tensor.matmul(
                                    d2[:, :], u_sb[:, j * P:(j + 1) * P],
                                    v_sb[:, :], start=True, stop=True)
                                ptj = pt_t[j][:, c0:c0 + half]
                                if direct_sqrt and not (diag_lo <= j < diag_hi):
                                    # off-diag d2 >= ~30: sqrt straight out
                                    # of PSUM (faster ACT access), bank held
                                    # until ACT consumes it (4 banks rotate).
                                    nc.scalar.activation(out=ptj, in_=d2[:, :],
                                                         func=AF.Sqrt)
                                else:
                                    # DVE relu-drain PSUM -> fp16 SBUF; frees
                                    # the bank without waiting on ACT.
                                    nc.vector.tensor_scalar_max(
                                        out=ptj, in0=d2[:, :], scalar1=0.0)
                                    nc.scalar.activation(out=ptj, in_=ptj,
                                                         func=AF.Sqrt)
                            for j in range(lo, hi):
                                ptj = pt_t[j][:, c0:c0 + half]
                                nc.scalar.activation(
                                    out=ptj, in_=ptj, func=AF.Exp,
                                    bias=qt_sb[:, j:j + 1], scale=-1.0,
                                )
                            for j in range(lo, hi):
                                if diag_lo <= j < diag_hi:
                                    it = j - diag_lo
                                    ptj = pt_t[j][:, c0:c0 + half]
                                    nc.gpsimd.affine_select(
                                        out=ptj, in_=ptj,
                                        compare_op=ALU.not_equal, fill=0.0,
                                        base=it * P, pattern=[[-1, half]],
                                        channel_multiplier=1)
                                    dtmp = dtmpp.tile([P, half], F16,
                                                      name="dtmp")
                                    nc.vector.tensor_scalar_mul(
                                        out=dtmp[:, :], in0=dmask1[it][:, :],
                                        scalar1=eq_sb[:, j:j + 1])
                                    nc.vector.tensor_add(ptj, ptj,
                                                         dtmp[:, :])
                            for j in range(lo, hi):
                                nc.gpsimd.tensor_add(
                                    dacc[:, :], dacc[:, :],
                                    pt_t[j][:, c0:c0 + half])

                        def g0_den_group(lo, hi):
                            for j in range(lo, hi):
                                e_sb = epool.tile([P, 512], F16, name="e_sb")
                                nc.sync.dma_start(
                                    out=e_sb[:, :],
                                    in_=e_d[j * P:(j + 1) * P, 0:512],
                                )
                                for it in range(it_n):
                                    nc.tensor.matmul(
                                        ps0_list[it][:, :],
                                        pt_t[j][:, c0 + it * P:
                                                c0 + (it + 1) * P],
                                        e_sb[:, :],
                                        start=(j == 0), stop=(j == jt_n - 1),
                                    )

                        n_groups = len(bounds) - 1
                        for gi in range(n_groups + lag):
                            if gi < n_groups:
                                phase1_group(bounds[gi], bounds[gi + 1])
                            if gi >= lag:
                                g0_den_group(bounds[gi - lag],
                                             bounds[gi - lag + 1])

                        # denominator reciprocal via DRAM-bounce redistribute
                        den_ps = mmpsum.tile([1, half], F32, name="den_ps",
                                             tag="mm")
                        nc.tensor.matmul(den_ps[:, :], ones_f32[:, :],
                                         dacc[:, :], start=True, stop=True)
                        den_row = smallp.tile([1, half], F32, name="den_row",
                                              tag="den_row")
                        nc.vector.tensor_copy(out=den_row[:, :],
                                              in_=den_ps[:, :])
                        den_dram = dramp.tile([1, half], F32, name="den_dram")
                        nc.sync.dma_start(out=den_dram[:, :],
                                          in_=den_row[:, :])
                        den_cols = smallp.tile([P, it_n], F32,
                                               name="den_cols", tag="den_c")
                        nc.sync.dma_start(
                            out=den_cols[:, :],
                            in_=den_dram.rearrange("o (t p) -> (o p) t", p=P),
                        )
                        nc.vector.reciprocal(
                            out=recip_all[:, h * it_n:(h + 1) * it_n],
                            in_=den_cols[:, :])

                        # g-block 0 epilogue for this half
                        for it in range(it_n):
                            o_sb = opool.tile([P, 512], F16 if out16 else F32,
                                              name="o_sb")
                            nc.vector.tensor_scalar_mul(
                                out=o_sb[:, :], in0=ps0_list[it][:, :],
                                scalar1=recip_all[:, h * it_n + it:
                                                  h * it_n + it + 1],
                            )
                            nc.sync.dma_start(
                                out=o_d[c0 + it * P:c0 + (it + 1) * P,
                                        0:512],
                                in_=o_sb[:, :],
                            )

                    # ---- g-blocks 1..3 full-width at full PE rate ----
                    for gb in range(1, gb_n):
                        ps_list = [
                            mmpsum.tile([P, 512], F32, name=f"ps{it}",
                                        tag="mm")
                            for it in range(it_full)
                        ]
                        for j in range(jt_n):
                            e_sb = epool.tile([P, 512], F16, name="e_sb")
                            nc.sync.dma_start(
                                out=e_sb[:, :],
                                in_=e_d[j * P:(j + 1) * P,
                                        gb * 512:(gb + 1) * 512],
                            )
                            for it in range(it_full):
                                nc.tensor.matmul(
                                    ps_list[it][:, :],
                                    pt_t[j][:, it * P:(it + 1) * P],
                                    e_sb[:, :],
                                    start=(j == 0), stop=(j == jt_n - 1),
                                )
                        for it in range(it_full):
                            o_sb = opool.tile([P, 512], F16 if out16 else F32,
                                              name="o_sb")
                            nc.vector.tensor_scalar_mul(
                                out=o_sb[:, :], in0=ps_list[it][:, :],
                                scalar1=recip_all[:, it:it + 1],
                            )
                            nc.sync.dma_start(
                                out=o_d[it * P:(it + 1) * P,
                                        gb * 512:(gb + 1) * 512],
                                in_=o_sb[:, :],
                            )

            if hw_loop:
                with tc.For_i(0, hw_loop, 1):
                    body()
            else:
                body()

    nc.compile()
    return nc


def make_in_maps(expression, encoding, quality, n_cores=N_CORES):
    b, n, d = encoding.shape
    g = expression.shape[2]
    rows = n // n_cores
    enc = np.ascontiguousarray(np.asarray(encoding, dtype=np.float32)[0])
    q = np.ascontiguousarray(np.asarray(quality, dtype=np.float32)[0, :, 0])
    expr16 = np.asarray(expression, dtype=np.float32)[0].astype(np.float16)

    x2 = (enc.astype(np.float64) ** 2).sum(axis=1).astype(np.float32)
    k = d + 2
    u = np.empty((k, n), np.float32)
    u[:d] = enc.T
    u[d] = x2
    u[d + 1] = 1.0
    v_all = np.empty((k, n), np.float32)
    v_all[:d] = -2.0 * enc.T
    v_all[d] = 1.0
    v_all[d + 1] = x2
    eq = np.exp(q).astype(np.float32)

    # Per-core j-rotation: roll the j-indexed inputs by -rows*c so each
    # core's diagonal block sits at the same compile-time j-tiles on every
    # core (softmax's sum over j is permutation invariant, so the output is
    # unchanged). v is i-indexed and is not rolled.
    in_maps = []
    for c in range(n_cores):
        sh = -(c * rows)
        in_maps.append({
            "u": np.ascontiguousarray(np.roll(u, sh, axis=1)),
            "v": np.ascontiguousarray(v_all[:, c * rows:(c + 1) * rows]),
            "qt": np.ascontiguousarray(np.roll(q, sh).reshape(n // P, P).T),
            "eq": np.ascontiguousarray(np.roll(eq, sh).reshape(n // P, P).T),
            "expr": np.ascontiguousarray(np.roll(expr16, sh, axis=0)),
        })
    return in_maps


_NC_CACHE = {}


def _get_nc(n, d, rows, g, repeat=1, hw_loop=0, **kw):
    key = (n, d, rows, g, repeat, hw_loop, tuple(sorted(kw.items())))
    if key not in _NC_CACHE:
        _NC_CACHE[key] = build_nc(n=n, d=d, rows=rows, g=g, repeat=repeat,
                                  hw_loop=hw_loop, **kw)
    return _NC_CACHE[key]


def kernel(expression, encoding, quality):
    from concourse.bass_utils import run_bass_kernel_spmd

    expression = np.asarray(expression)
    encoding = np.asarray(encoding)
    quality = np.asarray(quality)
    b, n, d = encoding.shape
    g = expression.shape[2]
    rows = n // N_CORES

    nc = _get_nc(n, d, rows, g)
    in_maps = make_in_maps(expression, encoding, quality)
    res = run_bass_kernel_spmd(nc, in_maps, core_ids=list(range(N_CORES)))
    out = np.concatenate([res.results[c]["out"] for c in range(N_CORES)], axis=0)
    return out[None].astype(np.float32)


# revision 13
# speedup vs baseline: 1.0216x; 1.0216x over previous
"""Trainium2 Bass kernel for nn_CellSmooth.

Computes: out = softmax(-cdist(enc, enc) + quality^T, axis=-1) @ expression
for B=1, N=8192, G=2048, D=64, sharded row-wise across 8 NeuronCores.

Design (per core, owning a 1024-row block of queries i):
  * d2[j, i] = |e_j|^2 + |e_i|^2 - 2 e_j.e_i is produced TRANSPOSED ([j, i]
    tiles, j on partitions) by a single K=66 augmented float32r matmul:
      U[:, j] = [enc_j (64), |e_j|^2, 1],  V[:, i] = [-2 enc_i (64), 1, |e_i|^2]
    U/V are built on the host (tiny). float32r runs the PE at full (bf16)
    rate for moving dims >= 256; measured accuracy ~7e-5 relative.
  * The d2_ii ~ 0 diagonal cannot survive float32r cancellation, so the host
    j-ROTATES the j-indexed inputs per core (roll by -1024*c): every core's
    diagonal then sits at compile-time-known j-tiles/positions (softmax's
    sum over j is permutation invariant, so the output is unchanged). Those
    positions are repaired after the exp with affine_select + a masked add
    of host-computed exp(quality) values.
  * P^T[j, i] = exp(quality_j - sqrt(d2)) is stored FP16 in 64 persistent
    [128, 1024] tiles covering the core's full 1024 i-rows (128 KB per
    partition): DVE relu-drains the d2 PSUM bank to the fp16 tile, ACT runs
    sqrt and exp in place (quality folds into the ACT exp bias). sqrt and
    exp live in different ACT table sets, so tiles go in groups of
    [sqrt x G, exp x G] to amortize the 1.3us table swaps. Measured fp16
    accuracy of the whole pipeline: 6e-4 relative.
  * The [j, i] P^T layout is exactly the stationary-operand layout the
    output matmul needs - the NxN matrix is never transposed. expression is
    converted to fp16 on the host: fp16 P x fp16 E matmuls run at the same
    1 cycle/row as f32r but halve SBUF and HBM traffic.
  * i is processed in two 512-column halves for phase 1 + g-block 0, each
    software-pipelined (d2/sqrt/exp groups chased two groups behind by the
    g-block-0 matmuls) so PE/ACT/DVE all stay busy; PSUM = 4 g0
    accumulators + 3-4 rotating d2 banks. denominator_i = sum_j P^T[j, i]
    accumulates on the otherwise-idle GPSIMD (acc += pt tile per j, after
    the diagonal fixup), then one f32 ones-stationary matmul reduces acc
    across partitions -> [1,512], redistributed to [128,4] through a DRAM
    bounce, then reciprocal. This keeps the PE free of 128 column-sum
    matmuls (~27us): on TRN2 every matmul pays a ~60ns stationary-load/
    drain overhead on top of its 512 moving cycles (measured 276ns vs the
    213ns compute for fp16, 253ns for f32r), which is why fewer, full-width
    matmuls win and why fp16's DMA savings roughly cancel f32r's cheaper
    self-loading stationaries elsewhere.
  * g-blocks 1..3 then run ONCE over the full 1024-wide P^T with all 8 PSUM
    banks as accumulators - expression streams from HBM once per g-block
    (fp16, 1KB/partition contiguous DMAs), 40 MB/core total vs 128 MB for
    the two-pass f32 version; out[i, g] = PSUM / den via DVE
    tensor_scalar multiply by 1/den, DMA out f32.
"""

import numpy as np

import concourse.bass as bass  # noqa: F401
import concourse.mybir as mybir
import concourse.tile as tile
from concourse import bacc

F32 = mybir.dt.float32
F32R = mybir.dt.float32r
F16 = mybir.dt.float16
AF = mybir.ActivationFunctionType
ALU = mybir.AluOpType

P = 128
N_CORES = 8


def build_nc(n=8192, d=64, rows=1024, g=2048, half=512, repeat=1, hw_loop=0,
             bounds_spec=(0, 4, 8, 16, 32, 48, 64), out16=False, lag=2):
    """Build the per-core Bass program (SPMD: all per-core variation is in
    the input data, never in the instruction stream).

    repeat>1 re-runs the body unrolled; hw_loop>0 wraps the body in a
    hardware For_i loop (constant instruction count). Both are for measuring
    steady-state HW exec time by wall-clock differencing in test.py.
    """
    jt_n = n // P          # j tiles (contraction for the output matmul)
    n_half = rows // half  # i column passes for phase 1 + g-block 0
    it_n = half // P       # i tiles per half
    it_full = rows // P    # i tiles full-width
    gb_n = g // 512        # g blocks
    k = d + 2              # augmented contraction for the d2 matmul

    nc = bacc.Bacc(None, target_bir_lowering=False)
    u_d = nc.dram_tensor("u", [k, n], F32, kind="ExternalInput")
    v_d = nc.dram_tensor("v", [k, rows], F32, kind="ExternalInput")
    qt_d = nc.dram_tensor("qt", [P, jt_n], F32, kind="ExternalInput")
    eq_d = nc.dram_tensor("eq", [P, jt_n], F32, kind="ExternalInput")
    e_d = nc.dram_tensor("expr", [n, g], F16, kind="ExternalInput")
    o_d = nc.dram_tensor("out", [rows, g], F16 if out16 else F32,
                         kind="ExternalOutput")

    with tile.TileContext(nc) as tc:
        with (
            tc.tile_pool(name="const", bufs=1) as constp,
            tc.tile_pool(name="vpool", bufs=2) as vpool,
            tc.tile_pool(name="ptpool", bufs=1) as ptpool,
            tc.tile_pool(name="estream", bufs=6) as epool,
            tc.tile_pool(name="ostage", bufs=4) as opool,
            tc.tile_pool(name="small", bufs=2) as smallp,
            tc.tile_pool(name="dtmp", bufs=1) as dtmpp,
            tc.tile_pool(name="denacc", bufs=2) as daccp,
            tc.tile_pool(name="mmpsum", bufs=8, space="PSUM") as mmpsum,
            tc.tile_pool(name="scratch", bufs=2, space="DRAM") as dramp,
        ):
            u_sb = constp.tile([k, n], F32R, name="u_sb")
            nc.sync.dma_start(out=u_sb, in_=u_d[:, :].bitcast(F32R))
            qt_sb = constp.tile([P, jt_n], F32, name="qt_sb")
            nc.sync.dma_start(out=qt_sb, in_=qt_d[:, :])
            eq_sb = constp.tile([P, jt_n], F32, name="eq_sb")
            nc.sync.dma_start(out=eq_sb, in_=eq_d[:, :])
            ones_f32 = constp.tile([P, 1], F32, name="ones_f32")
            nc.vector.memset(ones_f32, 1.0)
            ones_sb = constp.tile([P, 1], F16, name="ones_sb")
            nc.vector.tensor_copy(out=ones_sb[:, :], in_=ones_f32[:, :])
            # Diagonal masks: m1[it][p, c] = 1.0 iff c == it*128 + p.
            dmask1 = []
            for it in range(it_n):
                m1 = constp.tile([P, half], F16, name=f"dm1_{it}")
                nc.gpsimd.memset(m1, 0.0)
                nc.gpsimd.affine_select(
                    out=m1, in_=m1, compare_op=ALU.not_equal, fill=1.0,
                    base=it * P, pattern=[[-1, half]], channel_multiplier=1,
                )
                dmask1.append(m1)

            def body():
                for _ in range(repeat):
                    # Persistent fp16 P^T tiles spanning the full 1024 i.
                    pt_t = [
                        ptpool.tile([P, rows], F16, name=f"pt{j}",
                                    tag=f"pt{j}")
                        for j in range(jt_n)
                    ]
                    recip_all = smallp.tile([P, it_full], F32,
                                            name="recip_all", tag="recip")

                    for h in range(n_half):
                        c0 = h * half
                        v_sb = vpool.tile([k, half], F32R, name="v_sb")
                        nc.sync.dma_start(
                            out=v_sb,
                            in_=v_d[:, c0:c0 + half].bitcast(F32R))

                        # ---- phase 1 software-pipelined with g-block 0 +
                        # den. The PE issues in program order, so a plain
                        # phase1-then-phase2 order leaves the PE slot-blocked
                        # behind the ACT sqrt/exp pipeline for most of
                        # phase 1. Instead, emit phase-1 groups interleaved
                        # with the g-block-0 and denominator matmuls of the
                        # previous group.
                        diag_lo, diag_hi = h * it_n, (h + 1) * it_n
                        # Small leading groups fill the ACT->PE pipeline
                        # sooner; larger tail groups amortize ACT table swaps.
                        bounds = list(bounds_spec) if jt_n == 64 \
                            else list(range(0, jt_n + 1, 8))
                        ps0_list = [
                            mmpsum.tile([P, 512], F32, name=f"ps0_{it}",
                                        tag="mm")
                            for it in range(it_n)
                        ]
                        # denominator accumulates on the idle GPSIMD
                        # (acc += pt tile per j), freeing the PE of 64
                        # column-sum matmuls; one f32 ones-matmul then
                        # reduces acc across partitions.
                        dacc = daccp.tile([P, half], F32, name="dacc")
                        nc.gpsimd.memset(dacc, 0.0)

                        def phase1_group(lo, hi):
                            for j in range(lo, hi):
                                d2 = mmpsum.tile([P, half], F32, name="d2",
                                                 tag="mm")
                                nc.tensor.matmul(
                                    d2[:, :], u_sb[:, j * P:(j + 1) * P],
                                    v_sb[:, :], start=True, stop=True)
                                ptj = pt_t[j][:, c0:c0 + half]
                                # DVE relu-drain PSUM -> fp16 SBUF; frees the
                                # bank without waiting on the ACT pipeline.
                                nc.vector.tensor_scalar_max(
                                    out=ptj, in0=d2[:, :], scalar1=0.0)
                                nc.scalar.activation(out=ptj, in_=ptj,
                                                     func=AF.Sqrt)
                            for j in range(lo, hi):
                                ptj = pt_t[j][:, c0:c0 + half]
                                nc.scalar.activation(
                                    out=ptj, in_=ptj, func=AF.Exp,
                                    bias=qt_sb[:, j:j + 1], scale=-1.0,
                                )
                            for j in range(lo, hi):
                                if diag_lo <= j < diag_hi:
                                    it = j - diag_lo
                                    ptj = pt_t[j][:, c0:c0 + half]
                                    nc.gpsimd.affine_select(
                                        out=ptj, in_=ptj,
                                        compare_op=ALU.not_equal, fill=0.0,
                                        base=it * P, pattern=[[-1, half]],
                                        channel_multiplier=1)
                                    dtmp = dtmpp.tile([P, half], F16,
                                                      name="dtmp")
                                    nc.vector.tensor_scalar_mul(
                                        out=dtmp[:, :], in0=dmask1[it][:, :],
                                        scalar1=eq_sb[:, j:j + 1])
                                    nc.vector.tensor_add(ptj, ptj,
                                                         dtmp[:, :])
                            for j in range(lo, hi):
                                nc.gpsimd.tensor_add(
                                    dacc[:, :], dacc[:, :],
                                    pt_t[j][:, c0:c0 + half])

                        def g0_den_group(lo, hi):
                            for j in range(lo, hi):
                                e_sb = epool.tile([P, 512], F16, name="e_sb")
                                nc.sync.dma_start(
                                    out=e_sb[:, :],
                                    in_=e_d[j * P:(j + 1) * P, 0:512],
                                )
                                for it in range(it_n):
                                    nc.tensor.matmul(
                                        ps0_list[it][:, :],
                                        pt_t[j][:, c0 + it * P:
                                                c0 + (it + 1) * P],
                                        e_sb[:, :],
                                        start=(j == 0), stop=(j == jt_n - 1),
                                    )

                        n_groups = len(bounds) - 1
                        for gi in range(n_groups + lag):
                            if gi < n_groups:
                                phase1_group(bounds[gi], bounds[gi + 1])
                            if gi >= lag:
                                g0_den_group(bounds[gi - lag],
                                             bounds[gi - lag + 1])

                        # denominator reciprocal via DRAM-bounce redistribute
                        den_ps = mmpsum.tile([1, half], F32, name="den_ps",
                                             tag="mm")
                        nc.tensor.matmul(den_ps[:, :], ones_f32[:, :],
                                         dacc[:, :], start=True, stop=True)
                        den_row = smallp.tile([1, half], F32, name="den_row",
                                              tag="den_row")
                        nc.vector.tensor_copy(out=den_row[:, :],
                                              in_=den_ps[:, :])
                        den_dram = dramp.tile([1, half], F32, name="den_dram")
                        nc.sync.dma_start(out=den_dram[:, :],
                                          in_=den_row[:, :])
                        den_cols = smallp.tile([P, it_n], F32,
                                               name="den_cols", tag="den_c")
                        nc.sync.dma_start(
                            out=den_cols[:, :],
                            in_=den_dram.rearrange("o (t p) -> (o p) t", p=P),
                        )
                        nc.vector.reciprocal(
                            out=recip_all[:, h * it_n:(h + 1) * it_n],
                            in_=den_cols[:, :])

                        # g-block 0 epilogue for this half
                        for it in range(it_n):
                            o_sb = opool.tile([P, 512], F16 if out16 else F32,
                                              name="o_sb")
                            nc.vector.tensor_scalar_mul(
                                out=o_sb[:, :], in0=ps0_list[it][:, :],
                                scalar1=recip_all[:, h * it_n + it:
                                                  h * it_n + it + 1],
                            )
                            nc.sync.dma_start(
                                out=o_d[c0 + it * P:c0 + (it + 1) * P,
                                        0:512],
                                in_=o_sb[:, :],
                            )

                    # ---- g-blocks 1..3 full-width at full PE rate ----
                    for gb in range(1, gb_n):
                        ps_list = [
                            mmpsum.tile([P, 512], F32, name=f"ps{it}",
                                        tag="mm")
                            for it in range(it_full)
                        ]
                        for j in range(jt_n):
                            e_sb = epool.tile([P, 512], F16, name="e_sb")
                            nc.sync.dma_start(
                                out=e_sb[:, :],
                                in_=e_d[j * P:(j + 1) * P,
                                        gb * 512:(gb + 1) * 512],
                            )
                            for it in range(it_full):
                                nc.tensor.matmul(
                                    ps_list[it][:, :],
                                    pt_t[j][:, it * P:(it + 1) * P],
                                    e_sb[:, :],
                                    start=(j == 0), stop=(j == jt_n - 1),
                                )
                        for it in range(it_full):
                            o_sb = opool.tile([P, 512], F16 if out16 else F32,
                                              name="o_sb")
                            nc.vector.tensor_scalar_mul(
                                out=o_sb[:, :], in0=ps_list[it][:, :],
                                scalar1=recip_all[:, it:it + 1],
                            )
                            nc.sync.dma_start(
                                out=o_d[it * P:(it + 1) * P,
                                        gb * 512:(gb + 1) * 512],
                                in_=o_sb[:, :],
                            )

            if hw_loop:
                with tc.For_i(0, hw_loop, 1):
                    body()
            else:
                body()

    nc.compile()
    return nc


def make_in_maps(expression, encoding, quality, n_cores=N_CORES):
    b, n, d = encoding.shape
    g = expression.shape[2]
    rows = n // n_cores
    enc = np.ascontiguousarray(np.asarray(encoding, dtype=np.float32)[0])
    q = np.ascontiguousarray(np.asarray(quality, dtype=np.float32)[0, :, 0])
    expr16 = np.asarray(expression, dtype=np.float32)[0].astype(np.float16)

    x2 = (enc.astype(np.float64) ** 2).sum(axis=1).astype(np.float32)
    k = d + 2
    u = np.empty((k, n), np.float32)
    u[:d] = enc.T
    u[d] = x2
    u[d + 1] = 1.0
    v_all = np.empty((k, n), np.float32)
    v_all[:d] = -2.0 * enc.T
    v_all[d] = 1.0
    v_all[d + 1] = x2
    eq = np.exp(q).astype(np.float32)

    # Per-core j-rotation: roll the j-indexed inputs by -rows*c so each
    # core's diagonal block sits at the same compile-time j-tiles on every
    # core (softmax's sum over j is permutation invariant, so the output is
    # unchanged). v is i-indexed and is not rolled.
    in_maps = []
    for c in range(n_cores):
        sh = -(c * rows)
        in_maps.append({
            "u": np.ascontiguousarray(np.roll(u, sh, axis=1)),
            "v": np.ascontiguousarray(v_all[:, c * rows:(c + 1) * rows]),
            "qt": np.ascontiguousarray(np.roll(q, sh).reshape(n // P, P).T),
            "eq": np.ascontiguousarray(np.roll(eq, sh).reshape(n // P, P).T),
            "expr": np.ascontiguousarray(np.roll(expr16, sh, axis=0)),
        })
    return in_maps


_NC_CACHE = {}


def _get_nc(n, d, rows, g, repeat=1, hw_loop=0, **kw):
    key = (n, d, rows, g, repeat, hw_loop, tuple(sorted(kw.items())))
    if key not in _NC_CACHE:
        _NC_CACHE[key] = build_nc(n=n, d=d, rows=rows, g=g, repeat=repeat,
                                  hw_loop=hw_loop, **kw)
    return _NC_CACHE[key]


def kernel(expression, encoding, quality):
    from concourse.bass_utils import run_bass_kernel_spmd

    expression = np.asarray(expression)
    encoding = np.asarray(encoding)
    quality = np.asarray(quality)
    b, n, d = encoding.shape
    g = expression.shape[2]
    rows = n // N_CORES

    nc = _get_nc(n, d, rows, g)
    in_maps = make_in_maps(expression, encoding, quality)
    res = run_bass_kernel_spmd(nc, in_maps, core_ids=list(range(N_CORES)))
    out = np.concatenate([res.results[c]["out"] for c in range(N_CORES)], axis=0)
    return out[None].astype(np.float32)


# revision 14
# speedup vs baseline: 1.0350x; 1.0131x over previous
"""Trainium2 Bass kernel for nn_CellSmooth.

Computes: out = softmax(-cdist(enc, enc) + quality^T, axis=-1) @ expression
for B=1, N=8192, G=2048, D=64, sharded row-wise across 8 NeuronCores.

Design (per core, owning a 1024-row block of queries i):
  * d2[j, i] = |e_j|^2 + |e_i|^2 - 2 e_j.e_i is produced TRANSPOSED ([j, i]
    tiles, j on partitions) by a single K=66 augmented float32r matmul:
      U[:, j] = [enc_j (64), |e_j|^2, 1],  V[:, i] = [-2 enc_i (64), 1, |e_i|^2]
    U/V are built on the host (tiny). float32r runs the PE at full (bf16)
    rate for moving dims >= 256; measured accuracy ~7e-5 relative.
  * The d2_ii ~ 0 diagonal cannot survive float32r cancellation, so the host
    j-ROTATES the j-indexed inputs per core (roll by -1024*c): every core's
    diagonal then sits at compile-time-known j-tiles/positions (softmax's
    sum over j is permutation invariant, so the output is unchanged). Those
    positions are repaired after the exp with affine_select + a masked add
    of host-computed exp(quality) values.
  * P^T[j, i] = exp(quality_j - sqrt(d2)) is stored FP16 in 64 persistent
    [128, 1024] tiles covering the core's full 1024 i-rows (128 KB per
    partition): DVE relu-drains the d2 PSUM bank to the fp16 tile, ACT runs
    sqrt and exp in place (quality folds into the ACT exp bias). sqrt and
    exp live in different ACT table sets, so tiles go in groups of
    [sqrt x G, exp x G] to amortize the 1.3us table swaps. Measured fp16
    accuracy of the whole pipeline: 6e-4 relative.
  * The [j, i] P^T layout is exactly the stationary-operand layout the
    output matmul needs - the NxN matrix is never transposed. expression is
    converted to fp16 on the host: fp16 P x fp16 E matmuls run at the same
    1 cycle/row as f32r but halve SBUF and HBM traffic.
  * i is processed in two 512-column halves for phase 1 + g-block 0, each
    software-pipelined (d2/sqrt/exp groups chased two groups behind by the
    g-block-0 matmuls) so PE/ACT/DVE all stay busy; PSUM = 4 g0
    accumulators + 3-4 rotating d2 banks. denominator_i = sum_j P^T[j, i]
    accumulates on the otherwise-idle GPSIMD (acc += pt tile per j, after
    the diagonal fixup), then one f32 ones-stationary matmul reduces acc
    across partitions -> [1,512], redistributed to [128,4] through a DRAM
    bounce, then reciprocal. This keeps the PE free of 128 column-sum
    matmuls (~27us): on TRN2 every matmul pays a ~60ns stationary-load/
    drain overhead on top of its 512 moving cycles (measured 276ns vs the
    213ns compute for fp16, 253ns for f32r), which is why fewer, full-width
    matmuls win and why fp16's DMA savings roughly cancel f32r's cheaper
    self-loading stationaries elsewhere.
  * g-blocks 1..3 then run ONCE over the full 1024-wide P^T with all 8 PSUM
    banks as accumulators - expression streams from HBM once per g-block
    (fp16, 1KB/partition contiguous DMAs), 40 MB/core total vs 128 MB for
    the two-pass f32 version; out[i, g] = PSUM / den via DVE
    tensor_scalar multiply by 1/den, DMA out f32.
"""

import numpy as np

import concourse.bass as bass  # noqa: F401
import concourse.mybir as mybir
import concourse.tile as tile
from concourse import bacc

F32 = mybir.dt.float32
F32R = mybir.dt.float32r
F16 = mybir.dt.float16
AF = mybir.ActivationFunctionType
ALU = mybir.AluOpType

P = 128
N_CORES = 8


def build_nc(n=8192, d=64, rows=1024, g=2048, half=512, repeat=1, hw_loop=0,
             bounds_spec=(0, 4, 8, 16, 32, 48, 64), out16=False, lag=2):
    """Build the per-core Bass program (SPMD: all per-core variation is in
    the input data, never in the instruction stream).

    repeat>1 re-runs the body unrolled; hw_loop>0 wraps the body in a
    hardware For_i loop (constant instruction count). Both are for measuring
    steady-state HW exec time by wall-clock differencing in test.py.
    """
    jt_n = n // P          # j tiles (contraction for the output matmul)
    n_half = rows // half  # i column passes for phase 1 + g-block 0
    it_n = half // P       # i tiles per half
    it_full = rows // P    # i tiles full-width
    gb_n = g // 512        # g blocks
    k = d + 2              # augmented contraction for the d2 matmul

    nc = bacc.Bacc(None, target_bir_lowering=False)
    u_d = nc.dram_tensor("u", [k, n], F32, kind="ExternalInput")
    v_d = nc.dram_tensor("v", [k, rows], F32, kind="ExternalInput")
    qt_d = nc.dram_tensor("qt", [P, jt_n], F32, kind="ExternalInput")
    eq_d = nc.dram_tensor("eq", [P, jt_n], F32, kind="ExternalInput")
    e_d = nc.dram_tensor("expr", [n, g], F16, kind="ExternalInput")
    o_d = nc.dram_tensor("out", [rows, g], F16 if out16 else F32,
                         kind="ExternalOutput")

    with tile.TileContext(nc) as tc:
        with (
            tc.tile_pool(name="const", bufs=1) as constp,
            tc.tile_pool(name="vpool", bufs=2) as vpool,
            tc.tile_pool(name="ptpool", bufs=1) as ptpool,
            tc.tile_pool(name="estream", bufs=6) as epool,
            tc.tile_pool(name="ostage", bufs=8) as opool,
            tc.tile_pool(name="small", bufs=2) as smallp,
            tc.tile_pool(name="dtmp", bufs=1) as dtmpp,
            tc.tile_pool(name="denacc", bufs=2) as daccp,
            tc.tile_pool(name="mmpsum", bufs=8, space="PSUM") as mmpsum,
            tc.tile_pool(name="scratch", bufs=2, space="DRAM") as dramp,
        ):
            u_sb = constp.tile([k, n], F32R, name="u_sb")
            nc.sync.dma_start(out=u_sb, in_=u_d[:, :].bitcast(F32R))
            qt_sb = constp.tile([P, jt_n], F32, name="qt_sb")
            nc.sync.dma_start(out=qt_sb, in_=qt_d[:, :])
            eq_sb = constp.tile([P, jt_n], F32, name="eq_sb")
            nc.sync.dma_start(out=eq_sb, in_=eq_d[:, :])
            ones_f32 = constp.tile([P, 1], F32, name="ones_f32")
            nc.vector.memset(ones_f32, 1.0)
            ones_sb = constp.tile([P, 1], F16, name="ones_sb")
            nc.vector.tensor_copy(out=ones_sb[:, :], in_=ones_f32[:, :])
            # Diagonal masks: m1[it][p, c] = 1.0 iff c == it*128 + p.
            dmask1 = []
            for it in range(it_n):
                m1 = constp.tile([P, half], F16, name=f"dm1_{it}")
                nc.gpsimd.memset(m1, 0.0)
                nc.gpsimd.affine_select(
                    out=m1, in_=m1, compare_op=ALU.not_equal, fill=1.0,
                    base=it * P, pattern=[[-1, half]], channel_multiplier=1,
                )
                dmask1.append(m1)

            def body():
                for _ in range(repeat):
                    # Persistent fp16 P^T tiles spanning the full 1024 i.
                    pt_t = [
                        ptpool.tile([P, rows], F16, name=f"pt{j}",
                                    tag=f"pt{j}")
                        for j in range(jt_n)
                    ]
                    recip_all = smallp.tile([P, it_full], F32,
                                            name="recip_all", tag="recip")

                    for h in range(n_half):
                        c0 = h * half
                        v_sb = vpool.tile([k, half], F32R, name="v_sb")
                        nc.sync.dma_start(
                            out=v_sb,
                            in_=v_d[:, c0:c0 + half].bitcast(F32R))

                        # ---- phase 1 software-pipelined with g-block 0 +
                        # den. The PE issues in program order, so a plain
                        # phase1-then-phase2 order leaves the PE slot-blocked
                        # behind the ACT sqrt/exp pipeline for most of
                        # phase 1. Instead, emit phase-1 groups interleaved
                        # with the g-block-0 and denominator matmuls of the
                        # previous group.
                        diag_lo, diag_hi = h * it_n, (h + 1) * it_n
                        # Small leading groups fill the ACT->PE pipeline
                        # sooner; larger tail groups amortize ACT table swaps.
                        bounds = list(bounds_spec) if jt_n == 64 \
                            else list(range(0, jt_n + 1, 8))
                        ps0_list = [
                            mmpsum.tile([P, 512], F32, name=f"ps0_{it}",
                                        tag="mm")
                            for it in range(it_n)
                        ]
                        # denominator accumulates on the idle GPSIMD
                        # (acc += pt tile per j), freeing the PE of 64
                        # column-sum matmuls; one f32 ones-matmul then
                        # reduces acc across partitions.
                        dacc = daccp.tile([P, half], F32, name="dacc")
                        nc.gpsimd.memset(dacc, 0.0)

                        def phase1_group(lo, hi):
                            for j in range(lo, hi):
                                d2 = mmpsum.tile([P, half], F32, name="d2",
                                                 tag="mm")
                                nc.tensor.matmul(
                                    d2[:, :], u_sb[:, j * P:(j + 1) * P],
                                    v_sb[:, :], start=True, stop=True)
                                ptj = pt_t[j][:, c0:c0 + half]
                                # DVE relu-drain PSUM -> fp16 SBUF; frees the
                                # bank without waiting on the ACT pipeline.
                                nc.vector.tensor_scalar_max(
                                    out=ptj, in0=d2[:, :], scalar1=0.0)
                                nc.scalar.activation(out=ptj, in_=ptj,
                                                     func=AF.Sqrt)
                            for j in range(lo, hi):
                                ptj = pt_t[j][:, c0:c0 + half]
                                nc.scalar.activation(
                                    out=ptj, in_=ptj, func=AF.Exp,
                                    bias=qt_sb[:, j:j + 1], scale=-1.0,
                                )
                            for j in range(lo, hi):
                                if diag_lo <= j < diag_hi:
                                    it = j - diag_lo
                                    ptj = pt_t[j][:, c0:c0 + half]
                                    nc.gpsimd.affine_select(
                                        out=ptj, in_=ptj,
                                        compare_op=ALU.not_equal, fill=0.0,
                                        base=it * P, pattern=[[-1, half]],
                                        channel_multiplier=1)
                                    dtmp = dtmpp.tile([P, half], F16,
                                                      name="dtmp")
                                    nc.vector.tensor_scalar_mul(
                                        out=dtmp[:, :], in0=dmask1[it][:, :],
                                        scalar1=eq_sb[:, j:j + 1])
                                    nc.vector.tensor_add(ptj, ptj,
                                                         dtmp[:, :])
                            for j in range(lo, hi):
                                nc.gpsimd.tensor_add(
                                    dacc[:, :], dacc[:, :],
                                    pt_t[j][:, c0:c0 + half])

                        def g0_den_group(lo, hi):
                            for j in range(lo, hi):
                                e_sb = epool.tile([P, 512], F16, name="e_sb")
                                nc.sync.dma_start(
                                    out=e_sb[:, :],
                                    in_=e_d[j * P:(j + 1) * P, 0:512],
                                )
                                for it in range(it_n):
                                    nc.tensor.matmul(
                                        ps0_list[it][:, :],
                                        pt_t[j][:, c0 + it * P:
                                                c0 + (it + 1) * P],
                                        e_sb[:, :],
                                        start=(j == 0), stop=(j == jt_n - 1),
                                    )

                        n_groups = len(bounds) - 1
                        for gi in range(n_groups + lag):
                            if gi < n_groups:
                                phase1_group(bounds[gi], bounds[gi + 1])
                            if gi >= lag:
                                g0_den_group(bounds[gi - lag],
                                             bounds[gi - lag + 1])

                        # denominator reciprocal via DRAM-bounce redistribute
                        den_ps = mmpsum.tile([1, half], F32, name="den_ps",
                                             tag="mm")
                        nc.tensor.matmul(den_ps[:, :], ones_f32[:, :],
                                         dacc[:, :], start=True, stop=True)
                        den_row = smallp.tile([1, half], F32, name="den_row",
                                              tag="den_row")
                        nc.vector.tensor_copy(out=den_row[:, :],
                                              in_=den_ps[:, :])
                        den_dram = dramp.tile([1, half], F32, name="den_dram")
                        nc.sync.dma_start(out=den_dram[:, :],
                                          in_=den_row[:, :])
                        den_cols = smallp.tile([P, it_n], F32,
                                               name="den_cols", tag="den_c")
                        nc.sync.dma_start(
                            out=den_cols[:, :],
                            in_=den_dram.rearrange("o (t p) -> (o p) t", p=P),
                        )
                        nc.vector.reciprocal(
                            out=recip_all[:, h * it_n:(h + 1) * it_n],
                            in_=den_cols[:, :])

                        # g-block 0 epilogue for this half
                        for it in range(it_n):
                            o_sb = opool.tile([P, 512], F16 if out16 else F32,
                                              name="o_sb")
                            nc.vector.tensor_scalar_mul(
                                out=o_sb[:, :], in0=ps0_list[it][:, :],
                                scalar1=recip_all[:, h * it_n + it:
                                                  h * it_n + it + 1],
                            )
                            nc.sync.dma_start(
                                out=o_d[c0 + it * P:c0 + (it + 1) * P,
                                        0:512],
                                in_=o_sb[:, :],
                            )

                    # ---- g-blocks 1..3 full-width at full PE rate ----
                    for gb in range(1, gb_n):
                        ps_list = [
                            mmpsum.tile([P, 512], F32, name=f"ps{it}",
                                        tag="mm")
                            for it in range(it_full)
                        ]
                        for j in range(jt_n):
                            e_sb = epool.tile([P, 512], F16, name="e_sb")
                            nc.sync.dma_start(
                                out=e_sb[:, :],
                                in_=e_d[j * P:(j + 1) * P,
                                        gb * 512:(gb + 1) * 512],
                            )
                            for it in range(it_full):
                                nc.tensor.matmul(
                                    ps_list[it][:, :],
                                    pt_t[j][:, it * P:(it + 1) * P],
                                    e_sb[:, :],
                                    start=(j == 0), stop=(j == jt_n - 1),
                                )
                        for it in range(it_full):
                            o_sb = opool.tile([P, 512], F16 if out16 else F32,
                                              name="o_sb")
                            nc.vector.tensor_scalar_mul(
                                out=o_sb[:, :], in0=ps_list[it][:, :],
                                scalar1=recip_all[:, it:it + 1],
                            )
                            nc.sync.dma_start(
                                out=o_d[it * P:(it + 1) * P,
                                        gb * 512:(gb + 1) * 512],
                                in_=o_sb[:, :],
                            )

            if hw_loop:
                with tc.For_i(0, hw_loop, 1):
                    body()
            else:
                body()

    nc.compile()
    return nc


def make_in_maps(expression, encoding, quality, n_cores=N_CORES):
    b, n, d = encoding.shape
    g = expression.shape[2]
    rows = n // n_cores
    enc = np.ascontiguousarray(np.asarray(encoding, dtype=np.float32)[0])
    q = np.ascontiguousarray(np.asarray(quality, dtype=np.float32)[0, :, 0])
    expr16 = np.asarray(expression, dtype=np.float32)[0].astype(np.float16)

    x2 = (enc.astype(np.float64) ** 2).sum(axis=1).astype(np.float32)
    k = d + 2
    u = np.empty((k, n), np.float32)
    u[:d] = enc.T
    u[d] = x2
    u[d + 1] = 1.0
    v_all = np.empty((k, n), np.float32)
    v_all[:d] = -2.0 * enc.T
    v_all[d] = 1.0
    v_all[d + 1] = x2
    eq = np.exp(q).astype(np.float32)

    # Per-core j-rotation: roll the j-indexed inputs by -rows*c so each
    # core's diagonal block sits at the same compile-time j-tiles on every
    # core (softmax's sum over j is permutation invariant, so the output is
    # unchanged). v is i-indexed and is not rolled.
    in_maps = []
    for c in range(n_cores):
        sh = -(c * rows)
        in_maps.append({
            "u": np.ascontiguousarray(np.roll(u, sh, axis=1)),
            "v": np.ascontiguousarray(v_all[:, c * rows:(c + 1) * rows]),
            "qt": np.ascontiguousarray(np.roll(q, sh).reshape(n // P, P).T),
            "eq": np.ascontiguousarray(np.roll(eq, sh).reshape(n // P, P).T),
            "expr": np.ascontiguousarray(np.roll(expr16, sh, axis=0)),
        })
    return in_maps


_NC_CACHE = {}


def _get_nc(n, d, rows, g, repeat=1, hw_loop=0, **kw):
    key = (n, d, rows, g, repeat, hw_loop, tuple(sorted(kw.items())))
    if key not in _NC_CACHE:
        _NC_CACHE[key] = build_nc(n=n, d=d, rows=rows, g=g, repeat=repeat,
                                  hw_loop=hw_loop, **kw)
    return _NC_CACHE[key]


def kernel(expression, encoding, quality):
    from concourse.bass_utils import run_bass_kernel_spmd

    expression = np.asarray(expression)
    encoding = np.asarray(encoding)
    quality = np.asarray(quality)
    b, n, d = encoding.shape
    g = expression.shape[2]
    rows = n // N_CORES

    nc = _get_nc(n, d, rows, g)
    in_maps = make_in_maps(expression, encoding, quality)
    res = run_bass_kernel_spmd(nc, in_maps, core_ids=list(range(N_CORES)))
    out = np.concatenate([res.results[c]["out"] for c in range(N_CORES)], axis=0)
    return out[None].astype(np.float32)
